# revision 20
# baseline (speedup 1.0000x reference)
"""Global-KNN GCN kernel for Trainium2 (8 NeuronCores, SPMD).

Device computes the full fp8 (e4m3, DoubleRow) pairwise score matrix --
the PE floor is 1 output column per cycle at 256-deep contraction, so
the kernel minimizes streamed columns: 128-row stationary tiles (full
PE width) with the 49th leftover row tile's columns split 8 ways across
cores. Per core: 6 own row tiles x 6272 cols + 784 cols of the shared
tile = 38,416 column-streams x 8 channel groups ~ 307k PE cycles.

The centered -0.5*||x_j||^2 ranking term is folded into the contraction
(channels 2046/2047 sacrificed: moving side carries a coarse+residual
fp8 split of the norm, stationary side carries (1,1)).

Top-k is hierarchical to keep the DVE off the critical path: scores
drain PSUM->SBUF bf16 in 1536-col batches (Scalar engine), two rounds
of halving tensor_tensor-max build groups of 4 columns, then one
MAX8/FIND_INDEX8 per 384-group chunk. Candidates per row: 5 chunks x 8
groups x 4 cols = 160 (own tiles); the shared tile gets 2x8 groups x 4
cols per core, merged across all 8 cores on host.

Host does the cheap part: expands group candidates, exact fp32 rescore,
top-9, drop self, and the two small GCN layers (sparse aggregation).
"""

import os
import sys

import numpy as np

if "/opt/trn_rl_repo" not in sys.path:
    sys.path.insert(0, "/opt/trn_rl_repo")

B, H, W, C = 32, 14, 14, 2048
N = B * H * W            # 6272 nodes
K = 8                    # neighbors (excluding self)
N_CORES = 8
RT = 128                 # rows per tile (full PE width)
NT = 6                   # own row tiles per core
OWN = NT * RT            # 768 own rows per core
SHROWS = N - N_CORES * OWN   # 128 shared rows (tile 48)
SH_W = N // N_CORES          # 784 shared-tile cols per core
KP = C // 256            # 8 channel pair-chunks (256 channels each)
GEN = 1536               # psum generation width (3 banks of 512)
NGEN = 4                 # full generations per tile
RUNT = N - NGEN * GEN    # 128 runt cols
NB = 512                 # matmul chunk (one psum bank)
SLOTS = NGEN + 1         # cand slots per own tile
NCHUNK = NT * SLOTS + 2  # 6 tiles x (4 gens + runt) + 2 shared

LAST_EXEC_NS = None
LAST_KNN = None
_PROG = None


def _build_program():
    from concourse import bacc, tile, mybir

    f32 = mybir.dt.float32
    bf16 = mybir.dt.bfloat16
    f8 = mybir.dt.float8e4
    u16 = mybir.dt.uint16

    nc = bacc.Bacc("TRN2", target_bir_lowering=False)
    x8 = nc.declare_dram_parameter("x8", [KP, 128, 2, N], f8, isOutput=False)
    xr7d = nc.declare_dram_parameter("xr7", [128, 2, OWN], f8, isOutput=False)
    xshd = nc.declare_dram_parameter("xsh", [128, 2, KP, RT], f8, isOutput=False)
    xmvd = nc.declare_dram_parameter("xmv", [128, 2, KP, SH_W], f8, isOutput=False)
    cand = nc.declare_dram_parameter("cand", [NCHUNK, 128, 16], u16, isOutput=True)

    Act = mybir.ActivationFunctionType
    DR = mybir.MatmulPerfMode.DoubleRow
    MAX = mybir.AluOpType.max

    with tile.TileContext(nc) as tc:
        with (
            tc.tile_pool(name="persist", bufs=1) as pp,
            tc.tile_pool(name="score", bufs=3) as scp,
            tc.tile_pool(name="p1", bufs=2) as p1p,
            tc.tile_pool(name="p2", bufs=2) as p2p,
            tc.tile_pool(name="stage", bufs=10) as sp,
            tc.tile_pool(name="psum", bufs=2, space="PSUM") as psp,
            tc.tile_pool(name="pssh", bufs=2, space="PSUM") as pshp,
        ):
            xs = [pp.tile([128, 2, N], f8, name=f"xs{kp}") for kp in range(KP)]
            xr7 = pp.tile([128, 2, OWN], f8)
            xsh = pp.tile([128, 2, KP, RT], f8)
            xmv = pp.tile([128, 2, KP, SH_W], f8)
            ssh = pp.tile([128, SH_W], bf16)

            # all loads on the single sync HW-DGE queue: a second concurrent
            # DMA stream into SBUF slows every matmul ~20% (SBUF write
            # contention with the PE's weight/moving fetch). Ordered so the
            # gen-0 sweep (needing only cols 0:1536 of each group) can start
            # almost immediately.
            for kp in range(KP - 1):
                nc.sync.dma_start(out=xs[kp][:, :, 0:GEN], in_=x8[kp, :, :, 0:GEN])
            nc.sync.dma_start(out=xr7[:], in_=xr7d[:])
            nc.sync.dma_start(out=xs[KP - 1][:, :, 0:GEN],
                              in_=x8[KP - 1, :, :, 0:GEN])
            nc.sync.dma_start(out=xmv[:], in_=xmvd[:])
            nc.sync.dma_start(out=xsh[:], in_=xshd[:])
            # rest of each channel group as ONE large background transfer on
            # the idle gpsimd queue: fewer concurrent-DMA interruptions of
            # the PE's SBUF fetches than many small slab loads
            for kp in range(KP):
                nc.gpsimd.dma_start(out=xs[kp][:, :, GEN:N],
                                    in_=x8[kp, :, :, GEN:N])

            def topk_chunk(src_ap, slot):
                """MAX8 + FIND_INDEX8 over src_ap -> cand[slot]."""
                stage = sp.tile([128, 16], u16, tag="st")
                nc.vector.max(stage[:, 0:8].bitcast(bf16), src_ap)
                nc.vector.max_index(stage[:, 8:16], stage[:, 0:8].bitcast(bf16),
                                    src_ap)
                nc.sync.dma_start(out=cand[slot], in_=stage[:, :])

            def own_gen(t, g):
                r0 = t * RT
                if g < NGEN:
                    width, g0 = GEN, g * GEN
                else:
                    width, g0 = RUNT, NGEN * GEN
                ps = psp.tile([128, GEN], f32, tag="ps", name=f"ps_{t}_{g}")

                def lhsT(kp):
                    return (xs[kp][:, :, r0:r0 + RT] if kp < KP - 1
                            else xr7[:, :, r0:r0 + RT])

                def mm(kp, j, jw):
                    nc.tensor.matmul(
                        ps[:, j:j + jw],
                        lhsT(kp),
                        xs[kp][:, :, g0 + j:g0 + j + jw],
                        start=(kp == 0), stop=(kp == KP - 1),
                        perf_mode=DR, skip_group_check=True,
                    )

                s = scp.tile([128, GEN], bf16, tag="s", name=f"s_{t}_{g}")
                # kp-outer so tile-0/gen-0 can start as soon as the first
                # channel-group's slab lands
                for kp in range(KP):
                    for j in range(0, width, NB):
                        mm(kp, j, min(NB, width - j))
                nc.scalar.activation(s[:, 0:width], ps[:, 0:width], Act.Copy)
                if g < NGEN:
                    h = width // 2
                    q = width // 4
                    p1 = p1p.tile([128, GEN // 2], bf16, tag="p1")
                    p2 = p2p.tile([128, GEN // 4], bf16, tag="p2")
                    nc.vector.tensor_tensor(p1[:, 0:h], s[:, 0:h],
                                            s[:, h:width], MAX)
                    nc.vector.tensor_tensor(p2[:, 0:q], p1[:, 0:q],
                                            p1[:, q:h], MAX)
                    topk_chunk(p2[:, 0:q], t * SLOTS + g)
                else:
                    # runt: top-8 of the 128 raw cols directly (short tail)
                    topk_chunk(s[:, 0:width], t * SLOTS + g)

            def shared_tile():
                for hh in range(2):
                    ps = pshp.tile([128, SH_W // 2], f32, tag="pssh")
                    for kp in range(KP):
                        nc.tensor.matmul(
                            ps[:, :],
                            xsh[:, :, kp, :],
                            xmv[:, :, kp, hh * 392:(hh + 1) * 392],
                            start=(kp == 0), stop=(kp == KP - 1),
                            perf_mode=DR, skip_group_check=True,
                        )
                    nc.scalar.activation(ssh[:, hh * 392:(hh + 1) * 392],
                                         ps[:, :], Act.Copy)
                p1 = p1p.tile([128, GEN // 2], bf16, tag="p1")
                p2 = p2p.tile([128, GEN // 4], bf16, tag="p2")
                nc.vector.tensor_tensor(p1[:, 0:392], ssh[:, 0:392],
                                        ssh[:, 392:784], MAX)
                nc.vector.tensor_tensor(p2[:, 0:196], p1[:, 0:196],
                                        p1[:, 196:392], MAX)
                topk_chunk(p2[:, 0:98], NT * SLOTS)
                topk_chunk(p2[:, 98:196], NT * SLOTS + 1)

            # gen-major sweeps: gen g of all 6 tiles needs only column slab g,
            # so the PE saturates while later slabs stream in.
            for t in range(NT):
                own_gen(t, 0)
            shared_tile()
            for g in range(1, NGEN + 1):
                for t in range(NT):
                    own_gen(t, g)
    nc.compile()
    return nc


def _knn_from_device(x_flat):
    """Run the SPMD program; return knn [N, K] int64 global indices."""
    global LAST_EXEC_NS, LAST_KNN, _PROG
    import ml_dtypes
    from concourse.bass_utils import run_bass_kernel_spmd

    if _PROG is None:
        _PROG = _build_program()

    xq8 = x_flat.astype(ml_dtypes.float8_e4m3)               # [N, C]
    sq = np.sum(x_flat * x_flat, axis=1, dtype=np.float32)
    nhc = -0.5 * (sq - sq.mean())
    a = nhc.astype(ml_dtypes.float8_e4m3)
    bres = (nhc - a.astype(np.float32)).astype(ml_dtypes.float8_e4m3)
    # x8 layout [kp, p, i, n]: channel = kp*256 + i*128 + p
    x8T = np.ascontiguousarray(xq8.T)                        # [C, N]
    x8 = np.ascontiguousarray(
        x8T.reshape(KP, 2, 128, N).transpose(0, 2, 1, 3))    # [kp, p, i, n]
    # fold the norm term into sacrificed channels 2046/2047 (kp=7, i=1,
    # p=126/127): moving side carries (a, b); stationary side carries (1, 1)
    x8[KP - 1, 126, 1, :] = a
    x8[KP - 1, 127, 1, :] = bres

    one8 = np.float32(1.0).astype(ml_dtypes.float8_e4m3)
    # shared-tile stationary: rows 6144.., same for all cores
    xsh = np.ascontiguousarray(
        x8[:, :, :, N_CORES * OWN:N].transpose(1, 2, 0, 3))  # [p, i, kp, n]
    xsh[126, 1, KP - 1, :] = one8
    xsh[127, 1, KP - 1, :] = one8

    in_maps = []
    for c in range(N_CORES):
        sh = c * OWN
        x8c = np.ascontiguousarray(np.roll(x8, -sh, axis=3))
        xr7 = np.ascontiguousarray(x8c[KP - 1, :, :, 0:OWN])
        xr7[126, 1, :] = one8
        xr7[127, 1, :] = one8
        # shared moving window: rotated cols [16c, 16c+784) = global
        # [784c, 784(c+1))
        xmv = np.ascontiguousarray(
            x8c[:, :, :, 16 * c:16 * c + SH_W].transpose(1, 2, 0, 3))
        in_maps.append({"x8": x8c, "xr7": xr7, "xsh": xsh, "xmv": xmv})
    res = run_bass_kernel_spmd(
        _PROG, in_maps, list(range(N_CORES)),
        trace=bool(os.environ.get("KNN_TRACE")),
    )
    if res.exec_time_ns is not None:
        LAST_EXEC_NS = res.exec_time_ns

    # decode candidates
    TOWN = NGEN * 8 * 4 + 8                                  # cols per own row
    own_cols = np.empty((N_CORES * OWN, TOWN), dtype=np.int64)
    sh_cols = np.empty((128, N_CORES * 2 * 8 * 4), dtype=np.int64)
    m4 = np.arange(4, dtype=np.int64)
    for c, r in enumerate(res.results):
        o = r["cand"].astype(np.int64)                       # [NCHUNK, 128, 16]
        # own tiles
        for t in range(NT):
            cols_t = []
            for g in range(NGEN):
                idx = o[t * SLOTS + g, :, 8:16]                  # [128, 8]
                rot = g * GEN + idx[:, :, None] + m4[None, None, :] * (GEN // 4)
                cols_t.append(rot.reshape(128, 32))
            cols_t.append(NGEN * GEN + o[t * SLOTS + NGEN, :, 8:16])   # runt: raw
            rot = np.concatenate(cols_t, axis=1)             # [128, 136]
            gcol = (rot + c * OWN) % N
            own_cols[c * OWN + t * RT:c * OWN + (t + 1) * RT] = gcol
        # shared
        sh = []
        for hh in range(2):
            idx = o[NT * SLOTS + hh, :, 8:16]
            w = hh * 98 + idx[:, :, None] + m4[None, None, :] * 196
            sh.append(w.reshape(128, 32))
        sh_cols[:, c * 64:(c + 1) * 64] = np.concatenate(sh, axis=1) + SH_W * c

    # exact fp32 rescore + top-9 + drop self
    knn = np.empty((N, K), dtype=np.int64)

    def pick(rows, cidx):
        nr = len(rows)
        ex = np.empty((nr, cidx.shape[1]), dtype=np.float32)
        BLK = 256
        for i0 in range(0, nr, BLK):
            i1 = min(nr, i0 + BLK)
            cn = cidx[i0:i1]
            xc = x_flat[cn]                                  # [b, T, C]
            ex[i0:i1] = np.einsum("bc,bkc->bk", x_flat[rows[i0:i1]], xc,
                                  dtype=np.float32) - 0.5 * sq[cn]
        order = np.argsort(-ex, axis=1, kind="stable")[:, :K + 2]
        top = np.take_along_axis(cidx, order, axis=1)        # [nr, K+2]
        out = np.empty((nr, K), dtype=np.int64)
        rv = rows[:, None]
        for i in range(nr):
            t = top[i]
            t = t[t != rows[i]]
            # dedup, preserving order (device can emit duplicate groups)
            _, ui = np.unique(t, return_index=True)
            t = t[np.sort(ui)]
            out[i] = t[:K]
        return out

    own_rows = np.arange(N_CORES * OWN)
    knn[own_rows] = pick(own_rows, own_cols)
    shr = np.arange(N_CORES * OWN, N)
    knn[shr] = pick(shr, sh_cols)
    LAST_KNN = knn
    return knn


def kernel(x, W1, b1, W2, b2):
    x = np.asarray(x, dtype=np.float32)
    W1 = np.asarray(W1, dtype=np.float32)
    b1 = np.asarray(b1, dtype=np.float32)
    W2 = np.asarray(W2, dtype=np.float32)
    b2 = np.asarray(b2, dtype=np.float32)

    xf = x.reshape(N, C)
    knn = _knn_from_device(xf)

    src = np.repeat(np.arange(N, dtype=np.int64), K)
    dst = knn.reshape(-1)
    loops = np.arange(N, dtype=np.int64)
    src = np.concatenate([src, loops])
    dst = np.concatenate([dst, loops])

    deg = np.bincount(dst, minlength=N).astype(np.float32)
    dinv = 1.0 / np.sqrt(np.maximum(deg, 1.0))
    norm = (dinv[src] * dinv[dst]).astype(np.float32)

    try:
        import scipy.sparse as sps
        A = sps.csr_matrix((norm, (dst, src)), shape=(N, N), dtype=np.float32)

        def agg(hw):
            return A @ hw
    except Exception:
        def agg(hw):
            out = np.zeros_like(hw)
            np.add.at(out, dst, hw[src] * norm[:, None])
            return out

    h1 = np.maximum(agg(xf @ W1) + b1, 0.0).astype(np.float32)
    h2 = np.maximum(agg(h1 @ W2) + b2, 0.0).astype(np.float32)
    return h2.reshape(B, H, W, W2.shape[1]).astype(np.float32)


# revision 21
# speedup vs baseline: 1.3513x; 1.3513x over previous
"""Global-KNN GCN kernel for Trainium2 (8 NeuronCores, SPMD).

Device computes the full fp8 (e4m3, DoubleRow) pairwise score matrix --
the PE floor is 1 output column per cycle at 256-deep contraction, so
the kernel minimizes streamed columns: 128-row stationary tiles (full
PE width) with the 49th leftover row tile's columns split 8 ways across
cores. Per core: 6 own row tiles x 6272 cols + 784 cols of the shared
tile = 38,416 column-streams x 8 channel groups ~ 307k PE cycles.

The centered -0.5*||x_j||^2 ranking term is folded into the contraction
(channels 2046/2047 sacrificed: moving side carries a coarse+residual
fp8 split of the norm, stationary side carries (1,1)).

Top-k is hierarchical to keep the DVE off the critical path: scores
drain PSUM->SBUF bf16 in 1536-col batches (Scalar engine), two rounds
of halving tensor_tensor-max build groups of 4 columns, then one
MAX8/FIND_INDEX8 per 384-group chunk. Candidates per row: 5 chunks x 8
groups x 4 cols = 160 (own tiles); the shared tile gets 2x8 groups x 4
cols per core, merged across all 8 cores on host.

Host does the cheap part: expands group candidates, exact fp32 rescore,
top-9, drop self, and the two small GCN layers (sparse aggregation).
"""

import os
import sys

import numpy as np

if "/opt/trn_rl_repo" not in sys.path:
    sys.path.insert(0, "/opt/trn_rl_repo")

B, H, W, C = 32, 14, 14, 2048
N = B * H * W            # 6272 nodes
K = 8                    # neighbors (excluding self)
N_CORES = 8
RT = 128                 # rows per tile (full PE width)
NT = 6                   # own row tiles per core
OWN = NT * RT            # 768 own rows per core
SHROWS = N - N_CORES * OWN   # 128 shared rows (tile 48)
SH_W = N // N_CORES          # 784 shared-tile cols per core
KP = C // 256            # 8 channel pair-chunks (256 channels each)
GEN = 1536               # psum generation width (3 banks of 512)
NGEN = 4                 # full generations per tile
RUNT = N - NGEN * GEN    # 128 runt cols
NB = 512                 # matmul chunk (one psum bank)
SLOTS = NGEN + 1         # cand slots per own tile
NCHUNK = NT * SLOTS + 2  # 6 tiles x (4 gens + runt) + 2 shared

LAST_EXEC_NS = None
LAST_KNN = None
_PROG = None


def _build_program():
    from concourse import bacc, tile, mybir

    f32 = mybir.dt.float32
    bf16 = mybir.dt.bfloat16
    f8 = mybir.dt.float8e4
    u16 = mybir.dt.uint16

    nc = bacc.Bacc("TRN2", target_bir_lowering=False)
    x8 = nc.declare_dram_parameter("x8", [KP, 128, 2, N], f8, isOutput=False)
    xr7d = nc.declare_dram_parameter("xr7", [128, 2, OWN], f8, isOutput=False)
    xshd = nc.declare_dram_parameter("xsh", [128, 2, KP, RT], f8, isOutput=False)
    xmvd = nc.declare_dram_parameter("xmv", [128, 2, KP, SH_W], f8, isOutput=False)
    cand = nc.declare_dram_parameter("cand", [NCHUNK, 128, 16], u16, isOutput=True)

    Act = mybir.ActivationFunctionType
    DR = mybir.MatmulPerfMode.DoubleRow
    MAX = mybir.AluOpType.max

    with tile.TileContext(nc) as tc:
        with (
            tc.tile_pool(name="persist", bufs=1) as pp,
            tc.tile_pool(name="score", bufs=3) as scp,
            tc.tile_pool(name="p1", bufs=2) as p1p,
            tc.tile_pool(name="p2", bufs=2) as p2p,
            tc.tile_pool(name="stage", bufs=10) as sp,
            tc.tile_pool(name="psum", bufs=2, space="PSUM") as psp,
            tc.tile_pool(name="pssh", bufs=2, space="PSUM") as pshp,
        ):
            xs = [pp.tile([128, 2, N], f8, name=f"xs{kp}") for kp in range(KP)]
            xr7 = pp.tile([128, 2, OWN], f8)
            xsh = pp.tile([128, 2, KP, RT], f8)
            xmv = pp.tile([128, 2, KP, SH_W], f8)
            ssh = pp.tile([128, SH_W], bf16)

            # all loads on the single sync HW-DGE queue: a second concurrent
            # DMA stream into SBUF slows every matmul ~20% (SBUF write
            # contention with the PE's weight/moving fetch). Ordered so the
            # gen-0 sweep (needing only cols 0:1536 of each group) can start
            # almost immediately.
            for kp in range(KP - 1):
                nc.sync.dma_start(out=xs[kp][:, :, 0:GEN], in_=x8[kp, :, :, 0:GEN])
            nc.sync.dma_start(out=xr7[:], in_=xr7d[:])
            nc.sync.dma_start(out=xs[KP - 1][:, :, 0:GEN],
                              in_=x8[KP - 1, :, :, 0:GEN])
            nc.sync.dma_start(out=xmv[:], in_=xmvd[:])
            nc.sync.dma_start(out=xsh[:], in_=xshd[:])
            for g in range(1, NGEN):
                c0 = g * GEN
                c1 = min((g + 1) * GEN + (RUNT if g == NGEN - 1 else 0), N)
                for kp in range(KP):
                    nc.sync.dma_start(out=xs[kp][:, :, c0:c1],
                                      in_=x8[kp, :, :, c0:c1])

            def topk_chunk(src_ap, slot):
                """MAX8 + FIND_INDEX8 over src_ap -> cand[slot]."""
                stage = sp.tile([128, 16], u16, tag="st")
                nc.vector.max(stage[:, 0:8].bitcast(bf16), src_ap)
                nc.vector.max_index(stage[:, 8:16], stage[:, 0:8].bitcast(bf16),
                                    src_ap)
                nc.sync.dma_start(out=cand[slot], in_=stage[:, :])

            def own_gen(t, g):
                r0 = t * RT
                if g < NGEN:
                    width, g0 = GEN, g * GEN
                else:
                    width, g0 = RUNT, NGEN * GEN
                ps = psp.tile([128, GEN], f32, tag="ps", name=f"ps_{t}_{g}")

                def lhsT(kp):
                    return (xs[kp][:, :, r0:r0 + RT] if kp < KP - 1
                            else xr7[:, :, r0:r0 + RT])

                def mm(kp, j, jw):
                    nc.tensor.matmul(
                        ps[:, j:j + jw],
                        lhsT(kp),
                        xs[kp][:, :, g0 + j:g0 + j + jw],
                        start=(kp == 0), stop=(kp == KP - 1),
                        perf_mode=DR, skip_group_check=True,
                    )

                s = scp.tile([128, GEN], bf16, tag="s", name=f"s_{t}_{g}")
                # kp-outer so tile-0/gen-0 can start as soon as the first
                # channel-group's slab lands
                for kp in range(KP):
                    for j in range(0, width, NB):
                        mm(kp, j, min(NB, width - j))
                nc.scalar.activation(s[:, 0:width], ps[:, 0:width], Act.Copy)
                if g < NGEN:
                    h = width // 2
                    q = width // 4
                    p1 = p1p.tile([128, GEN // 2], bf16, tag="p1")
                    p2 = p2p.tile([128, GEN // 4], bf16, tag="p2")
                    nc.vector.tensor_tensor(p1[:, 0:h], s[:, 0:h],
                                            s[:, h:width], MAX)
                    nc.vector.tensor_tensor(p2[:, 0:q], p1[:, 0:q],
                                            p1[:, q:h], MAX)
                    topk_chunk(p2[:, 0:q], t * SLOTS + g)
                else:
                    # runt: top-8 of the 128 raw cols directly (short tail)
                    topk_chunk(s[:, 0:width], t * SLOTS + g)

            def shared_tile():
                for hh in range(2):
                    ps = pshp.tile([128, SH_W // 2], f32, tag="pssh")
                    for kp in range(KP):
                        nc.tensor.matmul(
                            ps[:, :],
                            xsh[:, :, kp, :],
                            xmv[:, :, kp, hh * 392:(hh + 1) * 392],
                            start=(kp == 0), stop=(kp == KP - 1),
                            perf_mode=DR, skip_group_check=True,
                        )
                    nc.scalar.activation(ssh[:, hh * 392:(hh + 1) * 392],
                                         ps[:, :], Act.Copy)
                p1 = p1p.tile([128, GEN // 2], bf16, tag="p1")
                p2 = p2p.tile([128, GEN // 4], bf16, tag="p2")
                nc.vector.tensor_tensor(p1[:, 0:392], ssh[:, 0:392],
                                        ssh[:, 392:784], MAX)
                nc.vector.tensor_tensor(p2[:, 0:196], p1[:, 0:196],
                                        p1[:, 196:392], MAX)
                topk_chunk(p2[:, 0:98], NT * SLOTS)
                topk_chunk(p2[:, 98:196], NT * SLOTS + 1)

            # gen-major sweeps: gen g of all 6 tiles needs only column slab g,
            # so the PE saturates while later slabs stream in.
            for t in range(NT):
                own_gen(t, 0)
            shared_tile()
            for g in range(1, NGEN + 1):
                for t in range(NT):
                    own_gen(t, g)
    nc.compile()
    return nc


def _knn_from_device(x_flat):
    """Run the SPMD program; return knn [N, K] int64 global indices."""
    global LAST_EXEC_NS, LAST_KNN, _PROG
    import ml_dtypes
    from concourse.bass_utils import run_bass_kernel_spmd

    if _PROG is None:
        _PROG = _build_program()

    xq8 = x_flat.astype(ml_dtypes.float8_e4m3)               # [N, C]
    sq = np.sum(x_flat * x_flat, axis=1, dtype=np.float32)
    nhc = -0.5 * (sq - sq.mean())
    a = nhc.astype(ml_dtypes.float8_e4m3)
    bres = (nhc - a.astype(np.float32)).astype(ml_dtypes.float8_e4m3)
    # x8 layout [kp, p, i, n]: channel = kp*256 + i*128 + p
    x8T = np.ascontiguousarray(xq8.T)                        # [C, N]
    x8 = np.ascontiguousarray(
        x8T.reshape(KP, 2, 128, N).transpose(0, 2, 1, 3))    # [kp, p, i, n]
    # fold the norm term into sacrificed channels 2046/2047 (kp=7, i=1,
    # p=126/127): moving side carries (a, b); stationary side carries (1, 1)
    x8[KP - 1, 126, 1, :] = a
    x8[KP - 1, 127, 1, :] = bres

    one8 = np.float32(1.0).astype(ml_dtypes.float8_e4m3)
    # shared-tile stationary: rows 6144.., same for all cores
    xsh = np.ascontiguousarray(
        x8[:, :, :, N_CORES * OWN:N].transpose(1, 2, 0, 3))  # [p, i, kp, n]
    xsh[126, 1, KP - 1, :] = one8
    xsh[127, 1, KP - 1, :] = one8

    in_maps = []
    for c in range(N_CORES):
        sh = c * OWN
        x8c = np.ascontiguousarray(np.roll(x8, -sh, axis=3))
        xr7 = np.ascontiguousarray(x8c[KP - 1, :, :, 0:OWN])
        xr7[126, 1, :] = one8
        xr7[127, 1, :] = one8
        # shared moving window: rotated cols [16c, 16c+784) = global
        # [784c, 784(c+1))
        xmv = np.ascontiguousarray(
            x8c[:, :, :, 16 * c:16 * c + SH_W].transpose(1, 2, 0, 3))
        in_maps.append({"x8": x8c, "xr7": xr7, "xsh": xsh, "xmv": xmv})
    res = run_bass_kernel_spmd(
        _PROG, in_maps, list(range(N_CORES)),
        trace=bool(os.environ.get("KNN_TRACE")),
    )
    if res.exec_time_ns is not None:
        LAST_EXEC_NS = res.exec_time_ns

    # decode candidates
    TOWN = NGEN * 8 * 4 + 8                                  # cols per own row
    own_cols = np.empty((N_CORES * OWN, TOWN), dtype=np.int64)
    sh_cols = np.empty((128, N_CORES * 2 * 8 * 4), dtype=np.int64)
    m4 = np.arange(4, dtype=np.int64)
    for c, r in enumerate(res.results):
        o = r["cand"].astype(np.int64)                       # [NCHUNK, 128, 16]
        # own tiles
        for t in range(NT):
            cols_t = []
            for g in range(NGEN):
                idx = o[t * SLOTS + g, :, 8:16]                  # [128, 8]
                rot = g * GEN + idx[:, :, None] + m4[None, None, :] * (GEN // 4)
                cols_t.append(rot.reshape(128, 32))
            cols_t.append(NGEN * GEN + o[t * SLOTS + NGEN, :, 8:16])   # runt: raw
            rot = np.concatenate(cols_t, axis=1)             # [128, 136]
            gcol = (rot + c * OWN) % N
            own_cols[c * OWN + t * RT:c * OWN + (t + 1) * RT] = gcol
        # shared
        sh = []
        for hh in range(2):
            idx = o[NT * SLOTS + hh, :, 8:16]
            w = hh * 98 + idx[:, :, None] + m4[None, None, :] * 196
            sh.append(w.reshape(128, 32))
        sh_cols[:, c * 64:(c + 1) * 64] = np.concatenate(sh, axis=1) + SH_W * c

    # exact fp32 rescore + top-9 + drop self
    knn = np.empty((N, K), dtype=np.int64)

    def pick(rows, cidx):
        nr = len(rows)
        ex = np.empty((nr, cidx.shape[1]), dtype=np.float32)
        BLK = 256
        for i0 in range(0, nr, BLK):
            i1 = min(nr, i0 + BLK)
            cn = cidx[i0:i1]
            xc = x_flat[cn]                                  # [b, T, C]
            ex[i0:i1] = np.einsum("bc,bkc->bk", x_flat[rows[i0:i1]], xc,
                                  dtype=np.float32) - 0.5 * sq[cn]
        order = np.argsort(-ex, axis=1, kind="stable")[:, :K + 2]
        top = np.take_along_axis(cidx, order, axis=1)        # [nr, K+2]
        out = np.empty((nr, K), dtype=np.int64)
        rv = rows[:, None]
        for i in range(nr):
            t = top[i]
            t = t[t != rows[i]]
            # dedup, preserving order (device can emit duplicate groups)
            _, ui = np.unique(t, return_index=True)
            t = t[np.sort(ui)]
            out[i] = t[:K]
        return out

    own_rows = np.arange(N_CORES * OWN)
    knn[own_rows] = pick(own_rows, own_cols)
    shr = np.arange(N_CORES * OWN, N)
    knn[shr] = pick(shr, sh_cols)
    LAST_KNN = knn
    return knn


def kernel(x, W1, b1, W2, b2):
    x = np.asarray(x, dtype=np.float32)
    W1 = np.asarray(W1, dtype=np.float32)
    b1 = np.asarray(b1, dtype=np.float32)
    W2 = np.asarray(W2, dtype=np.float32)
    b2 = np.asarray(b2, dtype=np.float32)

    xf = x.reshape(N, C)
    knn = _knn_from_device(xf)

    src = np.repeat(np.arange(N, dtype=np.int64), K)
    dst = knn.reshape(-1)
    loops = np.arange(N, dtype=np.int64)
    src = np.concatenate([src, loops])
    dst = np.concatenate([dst, loops])

    deg = np.bincount(dst, minlength=N).astype(np.float32)
    dinv = 1.0 / np.sqrt(np.maximum(deg, 1.0))
    norm = (dinv[src] * dinv[dst]).astype(np.float32)

    try:
        import scipy.sparse as sps
        A = sps.csr_matrix((norm, (dst, src)), shape=(N, N), dtype=np.float32)

        def agg(hw):
            return A @ hw
    except Exception:
        def agg(hw):
            out = np.zeros_like(hw)
            np.add.at(out, dst, hw[src] * norm[:, None])
            return out

    h1 = np.maximum(agg(xf @ W1) + b1, 0.0).astype(np.float32)
    h2 = np.maximum(agg(h1 @ W2) + b2, 0.0).astype(np.float32)
    return h2.reshape(B, H, W, W2.shape[1]).astype(np.float32)


# revision 22
# speedup vs baseline: 1.8675x; 1.3820x over previous
"""Global-KNN GCN kernel for Trainium2 (8 NeuronCores, SPMD).

Device computes the full fp8 (e4m3, DoubleRow) pairwise score matrix --
the PE floor is 1 output column per cycle at 256-deep contraction, so
the kernel minimizes streamed columns: 128-row stationary tiles (full
PE width) with the 49th leftover row tile's columns split 8 ways across
cores. Per core: 6 own row tiles x 6272 cols + 784 cols of the shared
tile = 38,416 column-streams x 8 channel groups ~ 307k PE cycles.

The centered -0.5*||x_j||^2 ranking term is folded into the contraction
(channels 2046/2047 sacrificed: moving side carries a coarse+residual
fp8 split of the norm, stationary side carries (1,1)).

Top-k is hierarchical to keep the DVE off the critical path: scores
drain PSUM->SBUF bf16 in 1536-col generations (Scalar engine), two
rounds of halving tensor_tensor-max build groups of 4 columns, then one
MAX8/FIND_INDEX8 per 384-group chunk (runt: direct top-8 of 128 cols).
Candidates per row: 4x32 + 8 = 136 cols (own tiles); the shared tile
gets 2x8 groups x 4 cols per core, merged across all 8 cores on host.
Taking top-8 of group-maxes then rescoring every group member on host
guarantees the chunk's true top-8 columns are all candidates.

Host does the cheap part: expands group candidates, exact fp32 rescore,
top-9, drop self, and the two small GCN layers (sparse aggregation).
"""

import os
import sys

import numpy as np

if "/opt/trn_rl_repo" not in sys.path:
    sys.path.insert(0, "/opt/trn_rl_repo")

B, H, W, C = 32, 14, 14, 2048
N = B * H * W            # 6272 nodes
K = 8                    # neighbors (excluding self)
N_CORES = 8
RT = 128                 # rows per tile (full PE width)
NT = 6                   # own row tiles per core
OWN = NT * RT            # 768 own rows per core
SHROWS = N - N_CORES * OWN   # 128 shared rows (tile 48)
SH_W = N // N_CORES          # 784 shared-tile cols per core
KP = C // 256            # 8 channel pair-chunks (256 channels each)
GEN = 1536               # psum generation width (3 banks of 512)
NGEN = 4                 # full generations per tile
RUNT = N - NGEN * GEN    # 128 runt cols
NB = 512                 # matmul chunk (one psum bank)
SLOTS = NGEN + 1         # cand slots per own tile
NCHUNK = NT * SLOTS + 2  # 6 tiles x (4 gens + runt) + 2 shared

LAST_EXEC_NS = None
LAST_KNN = None
_PROG = None


def _build_program():
    from concourse import bacc, tile, mybir

    f32 = mybir.dt.float32
    bf16 = mybir.dt.bfloat16
    f8 = mybir.dt.float8e4
    u16 = mybir.dt.uint16

    nc = bacc.Bacc("TRN2", target_bir_lowering=False)
    x8 = nc.declare_dram_parameter("x8", [KP, 128, 2, N], f8, isOutput=False)
    xr7d = nc.declare_dram_parameter("xr7", [128, 2, OWN], f8, isOutput=False)
    xshd = nc.declare_dram_parameter("xsh", [128, 2, KP, RT], f8, isOutput=False)
    xmvd = nc.declare_dram_parameter("xmv", [128, 2, KP, SH_W], f8, isOutput=False)
    cand = nc.declare_dram_parameter("cand", [NCHUNK, 128, 16], u16, isOutput=True)

    Act = mybir.ActivationFunctionType
    DR = mybir.MatmulPerfMode.DoubleRow
    MAX = mybir.AluOpType.max

    with tile.TileContext(nc) as tc:
        with (
            tc.tile_pool(name="persist", bufs=1) as pp,
            tc.tile_pool(name="score", bufs=3) as scp,
            tc.tile_pool(name="p1", bufs=2) as p1p,
            tc.tile_pool(name="p2", bufs=2) as p2p,
            tc.tile_pool(name="stage", bufs=10) as sp,
            tc.tile_pool(name="psum", bufs=2, space="PSUM") as psp,
            tc.tile_pool(name="pssh", bufs=2, space="PSUM") as pshp,
        ):
            xs = [pp.tile([128, 2, N], f8, name=f"xs{kp}") for kp in range(KP)]
            xr7 = pp.tile([128, 2, OWN], f8)
            xsh = pp.tile([128, 2, KP, RT], f8)
            xmv = pp.tile([128, 2, KP, SH_W], f8)
            ssh = pp.tile([128, SH_W], bf16)

            # all loads on the single sync HW-DGE queue: a second concurrent
            # DMA stream into SBUF slows every matmul ~20% (SBUF write
            # contention with the PE's weight/moving fetch). Ordered so the
            # gen-0 sweep (needing only cols 0:1536 of each group) can start
            # almost immediately.
            for kp in range(KP - 1):
                nc.sync.dma_start(out=xs[kp][:, :, 0:GEN], in_=x8[kp, :, :, 0:GEN])
            nc.sync.dma_start(out=xr7[:], in_=xr7d[:])
            nc.sync.dma_start(out=xs[KP - 1][:, :, 0:GEN],
                              in_=x8[KP - 1, :, :, 0:GEN])
            nc.sync.dma_start(out=xmv[:], in_=xmvd[:])
            nc.sync.dma_start(out=xsh[:], in_=xshd[:])
            for g in range(1, NGEN):
                c0 = g * GEN
                c1 = min((g + 1) * GEN + (RUNT if g == NGEN - 1 else 0), N)
                for kp in range(KP):
                    nc.sync.dma_start(out=xs[kp][:, :, c0:c1],
                                      in_=x8[kp, :, :, c0:c1])

            def topk_chunk(src_ap, slot):
                """MAX8 + FIND_INDEX8 over src_ap -> cand[slot]."""
                stage = sp.tile([128, 16], u16, tag="st")
                nc.vector.max(stage[:, 0:8].bitcast(bf16), src_ap)
                nc.vector.max_index(stage[:, 8:16], stage[:, 0:8].bitcast(bf16),
                                    src_ap)
                nc.sync.dma_start(out=cand[slot], in_=stage[:, :])

            def own_gen(t, g):
                r0 = t * RT
                if g < NGEN:
                    width, g0 = GEN, g * GEN
                else:
                    width, g0 = RUNT, NGEN * GEN
                ps = psp.tile([128, GEN], f32, tag="ps", name=f"ps_{t}_{g}")

                def lhsT(kp):
                    return (xs[kp][:, :, r0:r0 + RT] if kp < KP - 1
                            else xr7[:, :, r0:r0 + RT])

                def mm(kp, j, jw):
                    nc.tensor.matmul(
                        ps[:, j:j + jw],
                        lhsT(kp),
                        xs[kp][:, :, g0 + j:g0 + j + jw],
                        start=(kp == 0), stop=(kp == KP - 1),
                        perf_mode=DR, skip_group_check=True,
                    )

                s = scp.tile([128, GEN], bf16, tag="s", name=f"s_{t}_{g}")
                # kp-outer so tile-0/gen-0 can start as soon as the first
                # channel-group's slab lands
                for kp in range(KP):
                    for j in range(0, width, NB):
                        mm(kp, j, min(NB, width - j))
                nc.scalar.activation(s[:, 0:width], ps[:, 0:width], Act.Copy)
                if g < NGEN:
                    h = width // 2
                    q = width // 4
                    p1 = p1p.tile([128, GEN // 2], bf16, tag="p1")
                    p2 = p2p.tile([128, GEN // 4], bf16, tag="p2")
                    nc.vector.tensor_tensor(p1[:, 0:h], s[:, 0:h],
                                            s[:, h:width], MAX)
                    nc.vector.tensor_tensor(p2[:, 0:q], p1[:, 0:q],
                                            p1[:, q:h], MAX)
                    topk_chunk(p2[:, 0:q], t * SLOTS + g)
                else:
                    # runt: top-8 of the 128 raw cols directly (short tail)
                    topk_chunk(s[:, 0:width], t * SLOTS + g)

            def shared_tile():
                for hh in range(2):
                    ps = pshp.tile([128, SH_W // 2], f32, tag="pssh")
                    for kp in range(KP):
                        nc.tensor.matmul(
                            ps[:, :],
                            xsh[:, :, kp, :],
                            xmv[:, :, kp, hh * 392:(hh + 1) * 392],
                            start=(kp == 0), stop=(kp == KP - 1),
                            perf_mode=DR, skip_group_check=True,
                        )
                    nc.scalar.activation(ssh[:, hh * 392:(hh + 1) * 392],
                                         ps[:, :], Act.Copy)
                p1 = p1p.tile([128, GEN // 2], bf16, tag="p1")
                p2 = p2p.tile([128, GEN // 4], bf16, tag="p2")
                nc.vector.tensor_tensor(p1[:, 0:392], ssh[:, 0:392],
                                        ssh[:, 392:784], MAX)
                nc.vector.tensor_tensor(p2[:, 0:196], p1[:, 0:196],
                                        p1[:, 196:392], MAX)
                topk_chunk(p2[:, 0:98], NT * SLOTS)
                topk_chunk(p2[:, 98:196], NT * SLOTS + 1)

            # gen-major sweeps: gen g of all 6 tiles needs only column slab g,
            # so the PE saturates while later slabs stream in.
            for t in range(NT):
                own_gen(t, 0)
            shared_tile()
            for g in range(1, NGEN + 1):
                for t in range(NT):
                    own_gen(t, g)
    nc.compile()
    return nc


def _knn_from_device(x_flat):
    """Run the SPMD program; return knn [N, K] int64 global indices."""
    global LAST_EXEC_NS, LAST_KNN, _PROG
    import ml_dtypes
    from concourse.bass_utils import run_bass_kernel_spmd

    if _PROG is None:
        _PROG = _build_program()

    xq8 = x_flat.astype(ml_dtypes.float8_e4m3)               # [N, C]
    sq = np.sum(x_flat * x_flat, axis=1, dtype=np.float32)
    nhc = -0.5 * (sq - sq.mean())
    a = nhc.astype(ml_dtypes.float8_e4m3)
    bres = (nhc - a.astype(np.float32)).astype(ml_dtypes.float8_e4m3)
    # x8 layout [kp, p, i, n]: channel = kp*256 + i*128 + p
    x8T = np.ascontiguousarray(xq8.T)                        # [C, N]
    x8 = np.ascontiguousarray(
        x8T.reshape(KP, 2, 128, N).transpose(0, 2, 1, 3))    # [kp, p, i, n]
    # fold the norm term into sacrificed channels 2046/2047 (kp=7, i=1,
    # p=126/127): moving side carries (a, b); stationary side carries (1, 1)
    x8[KP - 1, 126, 1, :] = a
    x8[KP - 1, 127, 1, :] = bres

    one8 = np.float32(1.0).astype(ml_dtypes.float8_e4m3)
    # shared-tile stationary: rows 6144.., same for all cores
    xsh = np.ascontiguousarray(
        x8[:, :, :, N_CORES * OWN:N].transpose(1, 2, 0, 3))  # [p, i, kp, n]
    xsh[126, 1, KP - 1, :] = one8
    xsh[127, 1, KP - 1, :] = one8

    in_maps = []
    for c in range(N_CORES):
        sh = c * OWN
        x8c = np.ascontiguousarray(np.roll(x8, -sh, axis=3))
        xr7 = np.ascontiguousarray(x8c[KP - 1, :, :, 0:OWN])
        xr7[126, 1, :] = one8
        xr7[127, 1, :] = one8
        # shared moving window: rotated cols [16c, 16c+784) = global
        # [784c, 784(c+1))
        xmv = np.ascontiguousarray(
            x8c[:, :, :, 16 * c:16 * c + SH_W].transpose(1, 2, 0, 3))
        in_maps.append({"x8": x8c, "xr7": xr7, "xsh": xsh, "xmv": xmv})
    res = run_bass_kernel_spmd(
        _PROG, in_maps, list(range(N_CORES)),
        trace=bool(os.environ.get("KNN_TRACE")),
    )
    if res.exec_time_ns is not None:
        LAST_EXEC_NS = res.exec_time_ns

    # decode candidates
    TOWN = NGEN * 8 * 4 + 8                                  # cols per own row
    own_cols = np.empty((N_CORES * OWN, TOWN), dtype=np.int64)
    sh_cols = np.empty((128, N_CORES * 2 * 8 * 4), dtype=np.int64)
    m4 = np.arange(4, dtype=np.int64)
    for c, r in enumerate(res.results):
        o = r["cand"].astype(np.int64)                       # [NCHUNK, 128, 16]
        # own tiles
        for t in range(NT):
            cols_t = []
            for g in range(NGEN):
                idx = o[t * SLOTS + g, :, 8:16]                  # [128, 8]
                rot = g * GEN + idx[:, :, None] + m4[None, None, :] * (GEN // 4)
                cols_t.append(rot.reshape(128, 32))
            cols_t.append(NGEN * GEN + o[t * SLOTS + NGEN, :, 8:16])   # runt: raw
            rot = np.concatenate(cols_t, axis=1)             # [128, 136]
            gcol = (rot + c * OWN) % N
            own_cols[c * OWN + t * RT:c * OWN + (t + 1) * RT] = gcol
        # shared
        sh = []
        for hh in range(2):
            idx = o[NT * SLOTS + hh, :, 8:16]
            w = hh * 98 + idx[:, :, None] + m4[None, None, :] * 196
            sh.append(w.reshape(128, 32))
        sh_cols[:, c * 64:(c + 1) * 64] = np.concatenate(sh, axis=1) + SH_W * c

    # exact fp32 rescore + top-9 + drop self
    knn = np.empty((N, K), dtype=np.int64)

    def pick(rows, cidx):
        nr = len(rows)
        ex = np.empty((nr, cidx.shape[1]), dtype=np.float32)
        BLK = 256
        for i0 in range(0, nr, BLK):
            i1 = min(nr, i0 + BLK)
            cn = cidx[i0:i1]
            xc = x_flat[cn]                                  # [b, T, C]
            ex[i0:i1] = np.einsum("bc,bkc->bk", x_flat[rows[i0:i1]], xc,
                                  dtype=np.float32) - 0.5 * sq[cn]
        order = np.argsort(-ex, axis=1, kind="stable")[:, :K + 2]
        top = np.take_along_axis(cidx, order, axis=1)        # [nr, K+2]
        out = np.empty((nr, K), dtype=np.int64)
        rv = rows[:, None]
        for i in range(nr):
            t = top[i]
            t = t[t != rows[i]]
            # dedup, preserving order (device can emit duplicate groups)
            _, ui = np.unique(t, return_index=True)
            t = t[np.sort(ui)]
            out[i] = t[:K]
        return out

    own_rows = np.arange(N_CORES * OWN)
    knn[own_rows] = pick(own_rows, own_cols)
    shr = np.arange(N_CORES * OWN, N)
    knn[shr] = pick(shr, sh_cols)
    LAST_KNN = knn
    return knn


def kernel(x, W1, b1, W2, b2):
    x = np.asarray(x, dtype=np.float32)
    W1 = np.asarray(W1, dtype=np.float32)
    b1 = np.asarray(b1, dtype=np.float32)
    W2 = np.asarray(W2, dtype=np.float32)
    b2 = np.asarray(b2, dtype=np.float32)

    xf = x.reshape(N, C)
    knn = _knn_from_device(xf)

    src = np.repeat(np.arange(N, dtype=np.int64), K)
    dst = knn.reshape(-1)
    loops = np.arange(N, dtype=np.int64)
    src = np.concatenate([src, loops])
    dst = np.concatenate([dst, loops])

    deg = np.bincount(dst, minlength=N).astype(np.float32)
    dinv = 1.0 / np.sqrt(np.maximum(deg, 1.0))
    norm = (dinv[src] * dinv[dst]).astype(np.float32)

    try:
        import scipy.sparse as sps
        A = sps.csr_matrix((norm, (dst, src)), shape=(N, N), dtype=np.float32)

        def agg(hw):
            return A @ hw
    except Exception:
        def agg(hw):
            out = np.zeros_like(hw)
            np.add.at(out, dst, hw[src] * norm[:, None])
            return out

    h1 = np.maximum(agg(xf @ W1) + b1, 0.0).astype(np.float32)
    h2 = np.maximum(agg(h1 @ W2) + b2, 0.0).astype(np.float32)
    return h2.reshape(B, H, W, W2.shape[1]).astype(np.float32)


# revision 23
# speedup vs baseline: 1.9218x; 1.0291x over previous
"""Global-KNN GCN kernel for Trainium2 — ring-half symmetric variant.

Scores are made SYMMETRIC by sacrificing 4 channels (2044..2047): the
moving side carries (1, 1, a_j, b_j) and the stationary side carries
(a_i, b_i, 1, 1), where a+b is a coarse+residual fp8 split of the
centered -0.5*||x||^2 term. Then s(i,j) = G_ij + n_i + n_j = s(j,i),
so a transposed score block ranks columns correctly for its rows.

Each core computes, per own row tile t (rotated cols, SPMD-identical):
only the forward ring window [128t, 128t+3328) — every unordered pair
lands in exactly one forward window (d <= 3200 forward, else reverse).
The computed block is also TRANSPOSED on the PE (128x128 bf16 blocks,
~128 cyc each) to serve as the mirror half for the column tiles; mirror
blocks aggregate in SBUF per target tile j and get the same hierarchical
pair-max top-8 treatment. Leftover tile 48 gets a 4096-wide window split
512/core, with its own transposes.

PE work: (6*3328 + 512) cols x 8 groups = 164k cyc + ~21k transpose cyc
~= 77us/core, vs 307k cyc (128us) for the full-matrix kernel.

Host: expand group candidates per target tile, exact fp32 rescore,
top-9, drop self, two GCN layers.
"""

import os
import sys

import numpy as np

if "/opt/trn_rl_repo" not in sys.path:
    sys.path.insert(0, "/opt/trn_rl_repo")

B, H, W, C = 32, 14, 14, 2048
N = B * H * W            # 6272 nodes
K = 8
N_CORES = 8
RT = 128
NT = 6                   # own row tiles per core
OWN = NT * RT            # 768
NTILES = N // RT         # 49
WIN = 3328               # forward window (26 blocks)
NBLK = WIN // RT         # 26
GENW = [1536, 1024, 768]
GEN0 = [0, 1536, 2560]
NGEN = 3
NB = 512
SH_WIN = 4096            # tile-48 window
SH_W = SH_WIN // N_CORES  # 512 per core
MAXJ = NT - 1 + NBLK - 1  # 30: highest mirror target (rotated tile idx)
NMIR = MAXJ + 1           # 31 mirror buffers
# cand slots: own-upper 6*3, mirror 31, shared-upper 1, shared-mirror 4
SLOT_UP = 0
SLOT_MIR = NT * NGEN          # 18
SLOT_SHU = SLOT_MIR + NMIR    # 49
SLOT_SHM = SLOT_SHU + 1       # 50
NCHUNK = SLOT_SHM + 4         # 54

LAST_EXEC_NS = None
LAST_KNN = None
_PROG = None


def _mir_trange(j):
    return max(0, j - (NBLK - 1)), min(NT - 1, j)


def _build_program():
    from concourse import bacc, tile, mybir

    f32 = mybir.dt.float32
    bf16 = mybir.dt.bfloat16
    f8 = mybir.dt.float8e4
    u16 = mybir.dt.uint16

    nc = bacc.Bacc("TRN2", target_bir_lowering=False)
    x8 = nc.declare_dram_parameter("x8", [8, 128, 2, N], f8, isOutput=False)
    xr7d = nc.declare_dram_parameter("xr7", [128, 2, OWN], f8, isOutput=False)
    xshd = nc.declare_dram_parameter("xsh", [128, 2, 8, RT], f8, isOutput=False)
    xmvd = nc.declare_dram_parameter("xmv", [128, 2, 8, SH_W], f8, isOutput=False)
    eyed = nc.declare_dram_parameter("eye", [128, 128], bf16, isOutput=False)
    cand = nc.declare_dram_parameter("cand", [NCHUNK, 128, 16], u16, isOutput=True)

    Act = mybir.ActivationFunctionType
    DR = mybir.MatmulPerfMode.DoubleRow
    MAX = mybir.AluOpType.max
    KP = 8

    with tile.TileContext(nc) as tc:
        with (
            tc.tile_pool(name="persist", bufs=1) as pp,
            tc.tile_pool(name="score", bufs=3) as scp,
            tc.tile_pool(name="p1", bufs=3) as p1p,
            tc.tile_pool(name="p2", bufs=3) as p2p,
            tc.tile_pool(name="stage", bufs=10) as sp,
            tc.tile_pool(name="psum", bufs=2, space="PSUM") as psp,
            tc.tile_pool(name="ptr", bufs=2, space="PSUM") as ptp,
        ):
            XCOLS = (NT - 1) * RT + WIN              # 3968 cols ever touched
            xs = [pp.tile([128, 2, XCOLS], f8, name=f"xs{kp}")
                  for kp in range(KP)]
            xr7 = pp.tile([128, 2, OWN], f8)
            xsh = pp.tile([128, 2, KP, RT], f8)
            xmv = pp.tile([128, 2, KP, SH_W], f8)
            eye = pp.tile([128, 128], bf16)
            ssh = pp.tile([128, SH_W], bf16)
            M = pp.tile([128, NMIR, OWN], bf16)      # mirror aggregation
            Msh = pp.tile([128, 4, 128], bf16)       # shared-tile mirrors

            # single sync HW-DGE queue for bulk loads (concurrent streams
            # contend with PE SBUF fetches); first slab per group first
            for kp in range(KP - 1):
                nc.sync.dma_start(out=xs[kp][:, :, 0:GENW[0]],
                                  in_=x8[kp, :, :, 0:GENW[0]])
            nc.sync.dma_start(out=xr7[:], in_=xr7d[:])
            nc.sync.dma_start(out=xs[KP - 1][:, :, 0:GENW[0]],
                              in_=x8[KP - 1, :, :, 0:GENW[0]])
            nc.sync.dma_start(out=eye[:], in_=eyed[:])
            nc.sync.dma_start(out=xmv[:], in_=xmvd[:])
            nc.sync.dma_start(out=xsh[:], in_=xshd[:])
            # remaining columns: window of last tile reaches 128*5+3328=3968;
            # shared window is served by xmv. Load [1536:3968) in two slabs.
            for c0, c1 in [(1536, 3072), (3072, 3968)]:
                for kp in range(KP):
                    nc.sync.dma_start(out=xs[kp][:, :, c0:c1],
                                      in_=x8[kp, :, :, c0:c1])

            def topk_chunk(src_ap, slot):
                stage = sp.tile([128, 16], u16, tag="st")
                nc.vector.max(stage[:, 0:8].bitcast(bf16), src_ap)
                nc.vector.max_index(stage[:, 8:16], stage[:, 0:8].bitcast(bf16),
                                    src_ap)
                nc.sync.dma_start(out=cand[slot], in_=stage[:, :])

            def hier_topk(src, width, slot, eng=None):
                """pair-max halving x2 then top-8 of width/4 groups.

                eng=nc.gpsimd offloads the halving to the idle Pool engine
                (used for mirror chains, which bunch near the end of the
                run when the DVE is busiest)."""
                eng = eng or nc.vector
                hf, qt = width // 2, width // 4
                p1 = p1p.tile([128, 768], bf16, tag="p1")
                p2 = p2p.tile([128, 384], bf16, tag="p2")
                eng.tensor_tensor(p1[:, 0:hf], src[:, 0:hf],
                                  src[:, hf:width], MAX)
                eng.tensor_tensor(p2[:, 0:qt], p1[:, 0:qt],
                                  p1[:, qt:hf], MAX)
                topk_chunk(p2[:, 0:qt], slot)

            mir_done = [0] * NMIR     # contributed-tile count per buffer

            def emit_transposes(t, g, s):
                """Transpose gen (t,g)'s 128-col blocks into mirror slots."""
                m0 = GEN0[g] // RT
                nblk = GENW[g] // RT
                done = []
                for b0 in range(0, nblk, 8):
                    bn = min(8, nblk - b0)
                    pt = ptp.tile([128, 1024], bf16, tag="pt")
                    for k in range(bn):
                        nc.tensor.transpose(
                            pt[:, k * 128:(k + 1) * 128],
                            s[:, (b0 + k) * RT:(b0 + k + 1) * RT],
                            eye[:],
                        )
                    j0 = t + m0 + b0
                    nc.scalar.activation(
                        M[:, j0:j0 + bn, t * RT:(t + 1) * RT],
                        pt[:, 0:bn * 128], Act.Copy)
                    done.extend(range(j0, j0 + bn))
                return done

            def flush_mirrors(newly):
                for j in newly:
                    mir_done[j] += 1
                for j in range(NMIR):
                    tmin, tmax = _mir_trange(j)
                    if mir_done[j] == tmax - tmin + 1:
                        mir_done[j] = -99  # emitted
                        hier_topk(M[:, j, tmin * RT:(tmax + 1) * RT],
                                  (tmax - tmin + 1) * RT, SLOT_MIR + j)

            def own_gen(t, g):
                r0 = t * RT
                width, g0 = GENW[g], GEN0[g]
                w0 = r0 + g0                       # window offset in rotated cols
                ps = psp.tile([128, 1536], f32, tag="ps", name=f"ps_{t}_{g}")
                for kp in range(KP):
                    lhsT = (xs[kp][:, :, r0:r0 + RT] if kp < KP - 1
                            else xr7[:, :, r0:r0 + RT])
                    for j in range(0, width, NB):
                        jw = min(NB, width - j)
                        nc.tensor.matmul(
                            ps[:, j:j + jw], lhsT,
                            xs[kp][:, :, w0 + j:w0 + j + jw],
                            start=(kp == 0), stop=(kp == KP - 1),
                            perf_mode=DR, skip_group_check=True,
                        )
                s = scp.tile([128, 1536], bf16, tag="s", name=f"s_{t}_{g}")
                nc.scalar.activation(s[:, 0:width], ps[:, 0:width], Act.Copy)
                hier_topk(s, width, SLOT_UP + t * NGEN + g)
                return s

            def shared_compute():
                """tile 48: this core's 512-col slice (matmul + upper topk)."""
                ps = psp.tile([128, 1536], f32, tag="ps", name="ps_sh")
                for kp in range(KP):
                    nc.tensor.matmul(
                        ps[:, 0:SH_W], xsh[:, :, kp, :], xmv[:, :, kp, :],
                        start=(kp == 0), stop=(kp == KP - 1),
                        perf_mode=DR, skip_group_check=True,
                    )
                nc.scalar.activation(ssh[:, :], ps[:, 0:SH_W], Act.Copy)
                hier_topk(ssh, SH_W, SLOT_SHU)

            def shared_transposes():
                pt = ptp.tile([128, 1024], bf16, tag="pt")
                for k in range(4):
                    nc.tensor.transpose(pt[:, k * 128:(k + 1) * 128],
                                        ssh[:, k * 128:(k + 1) * 128], eye[:])
                nc.scalar.activation(Msh[:, :, :], pt[:, 0:512], Act.Copy)
                for k in range(4):
                    hier_topk(Msh[:, k, :], 128, SLOT_SHM + k)

            # gen-major sweeps; transposes delayed one unit so the PE never
            # waits on the score drain. Last sweep runs t descending so the
            # final mirror buffers (which need tile 0's last blocks) finish
            # while earlier buffers' chains still overlap compute.
            units = [(t, g) for g in range(NGEN - 1) for t in range(NT)]
            units += [(t, NGEN - 1) for t in reversed(range(NT))]
            prev = None
            for n, (t, g) in enumerate(units):
                s = own_gen(t, g)
                if prev is not None:
                    flush_mirrors(emit_transposes(*prev))
                if (t, g) == (NT - 1, 0):
                    shared_compute()
                if (t, g) == (0, 1):
                    shared_transposes()
                prev = (t, g, s)

            flush_mirrors(emit_transposes(*prev))
    nc.compile()
    return nc


def _knn_from_device(x_flat):
    global LAST_EXEC_NS, LAST_KNN, _PROG
    import ml_dtypes
    from concourse.bass_utils import run_bass_kernel_spmd

    if _PROG is None:
        _PROG = _build_program()

    xq8 = x_flat.astype(ml_dtypes.float8_e4m3)
    sq = np.sum(x_flat * x_flat, axis=1, dtype=np.float32)
    nhc = -0.5 * (sq - sq.mean())
    a = nhc.astype(ml_dtypes.float8_e4m3)
    bres = (nhc - a.astype(np.float32)).astype(ml_dtypes.float8_e4m3)
    x8T = np.ascontiguousarray(xq8.T)
    x8 = np.ascontiguousarray(
        x8T.reshape(8, 2, 128, N).transpose(0, 2, 1, 3))     # [kp, p, i, n]
    one8 = np.float32(1.0).astype(ml_dtypes.float8_e4m3)
    # moving side channels 2044..2047 = (1, 1, a_j, b_j)
    x8[7, 124, 1, :] = one8
    x8[7, 125, 1, :] = one8
    x8[7, 126, 1, :] = a
    x8[7, 127, 1, :] = bres

    def stat_norm(panel, rows):
        """stationary channels 2044..2047 = (a_i, b_i, 1, 1)."""
        panel[124, 1, :] = a[rows]
        panel[125, 1, :] = bres[rows]
        panel[126, 1, :] = one8
        panel[127, 1, :] = one8

    xsh = np.ascontiguousarray(x8[:, :, :, N_CORES * OWN:N])  # [kp, p, i, 128]
    stat_norm(xsh[7], np.arange(N_CORES * OWN, N))
    xsh_in = np.ascontiguousarray(xsh.transpose(1, 2, 0, 3))  # [p,i,kp,n]

    eye = np.eye(128, dtype=ml_dtypes.bfloat16)

    in_maps = []
    for c in range(N_CORES):
        sh = c * OWN
        x8c = np.ascontiguousarray(np.roll(x8, -sh, axis=3))
        xr7 = np.ascontiguousarray(x8c[7][:, :, 0:OWN])
        stat_norm(xr7, (np.arange(OWN) + sh) % N)
        # shared moving slice: global cols [6144+512c, 6144+512c+512) mod N
        cols = (np.arange(SH_W) + N_CORES * OWN + SH_W * c) % N
        xmv = np.ascontiguousarray(
            x8[:, :, :, cols].transpose(1, 2, 0, 3))
        in_maps.append({"x8": x8c, "xr7": xr7, "xsh": xsh_in, "xmv": xmv,
                        "eye": eye})
    res = run_bass_kernel_spmd(
        _PROG, in_maps, list(range(N_CORES)),
        trace=bool(os.environ.get("KNN_TRACE")),
    )
    if res.exec_time_ns is not None:
        LAST_EXEC_NS = res.exec_time_ns

    # ---- decode: gather candidate columns per global row tile ----
    per_tile = [[] for _ in range(NTILES)]   # lists of [128, n] col arrays
    m4 = np.arange(4, dtype=np.int64)
    for c, r in enumerate(res.results):
        o = r["cand"].astype(np.int64)                   # [NCHUNK, 128, 16]
        # own upper: rows = tile 6c+t, cols rotated 128t+g0+group
        for t in range(NT):
            for g in range(NGEN):
                idx = o[SLOT_UP + t * NGEN + g, :, 8:16]
                w4 = GENW[g] // 4
                rot = t * RT + GEN0[g] + idx[:, :, None] + m4 * w4
                gcol = (rot.reshape(128, 32) + c * OWN) % N
                per_tile[(6 * c + t) % NTILES].append(gcol)
        # mirrors: buffer j rows = tile (6c+j) mod 49; cols = own rows
        for j in range(NMIR):
            tmin, tmax = _mir_trange(j)
            wj = (tmax - tmin + 1) * RT
            idx = o[SLOT_MIR + j, :, 8:16]
            w = idx[:, :, None] + m4 * (wj // 4)
            gcol = c * OWN + tmin * RT + w.reshape(128, 32)
            per_tile[(6 * c + j) % NTILES].append(gcol)
        # shared upper: rows = tile 48; cols = window slice
        idx = o[SLOT_SHU, :, 8:16]
        rot = idx[:, :, None] + m4 * (SH_W // 4)
        gcol = (N_CORES * OWN + SH_W * c + rot.reshape(128, 32)) % N
        per_tile[NTILES - 1].append(gcol)
        # shared mirrors: block k rows = tile (48 + 4c + k) mod 49
        for k in range(4):
            idx = o[SLOT_SHM + k, :, 8:16]
            w = idx[:, :, None] + m4 * 32
            gcol = N_CORES * OWN + w.reshape(128, 32)
            per_tile[(NTILES - 1 + 4 * c + k) % NTILES].append(gcol)

    knn = np.empty((N, K), dtype=np.int64)
    for T in range(NTILES):
        cidx = np.concatenate(per_tile[T], axis=1)       # [128, Tt]
        rows = np.arange(T * RT, (T + 1) * RT)
        xc = x_flat[cidx]                                # [128, Tt, C]
        ex = np.einsum("bc,bkc->bk", x_flat[rows], xc,
                       dtype=np.float32) - 0.5 * sq[cidx]
        order = np.argsort(-ex, axis=1, kind="stable")[:, :3 * (K + 1)]
        top = np.take_along_axis(cidx, order, axis=1)
        for i in range(RT):
            tt = top[i]
            tt = tt[tt != rows[i]]
            _, ui = np.unique(tt, return_index=True)
            tt = tt[np.sort(ui)]
            knn[rows[i]] = tt[:K]
    LAST_KNN = knn
    return knn


def kernel(x, W1, b1, W2, b2):
    x = np.asarray(x, dtype=np.float32)
    W1 = np.asarray(W1, dtype=np.float32)
    b1 = np.asarray(b1, dtype=np.float32)
    W2 = np.asarray(W2, dtype=np.float32)
    b2 = np.asarray(b2, dtype=np.float32)

    xf = x.reshape(N, C)
    knn = _knn_from_device(xf)

    src = np.repeat(np.arange(N, dtype=np.int64), K)
    dst = knn.reshape(-1)
    loops = np.arange(N, dtype=np.int64)
    src = np.concatenate([src, loops])
    dst = np.concatenate([dst, loops])

    deg = np.bincount(dst, minlength=N).astype(np.float32)
    dinv = 1.0 / np.sqrt(np.maximum(deg, 1.0))
    norm = (dinv[src] * dinv[dst]).astype(np.float32)

    try:
        import scipy.sparse as sps
        A = sps.csr_matrix((norm, (dst, src)), shape=(N, N), dtype=np.float32)

        def agg(hw):
            return A @ hw
    except Exception:
        def agg(hw):
            out = np.zeros_like(hw)
            np.add.at(out, dst, hw[src] * norm[:, None])
            return out

    h1 = np.maximum(agg(xf @ W1) + b1, 0.0).astype(np.float32)
    h2 = np.maximum(agg(h1 @ W2) + b2, 0.0).astype(np.float32)
    return h2.reshape(B, H, W, W2.shape[1]).astype(np.float32)


# revision 24
# speedup vs baseline: 1.9522x; 1.0158x over previous
"""Global-KNN GCN kernel for Trainium2 — ring-half symmetric variant.

Scores are made SYMMETRIC by sacrificing 4 channels (2044..2047): the
moving side carries (1, 1, a_j, b_j) and the stationary side carries
(a_i, b_i, 1, 1), where a+b is a coarse+residual fp8 split of the
centered -0.5*||x||^2 term. Then s(i,j) = G_ij + n_i + n_j = s(j,i),
so a transposed score block ranks columns correctly for its rows.

Each core computes, per own row tile t (rotated cols, SPMD-identical):
only the forward ring window [128t, 128t+3328) — every unordered pair
lands in exactly one forward window (d <= 3200 forward, else reverse).
The computed block is also TRANSPOSED on the PE (128x128 bf16 blocks,
~128 cyc each) to serve as the mirror half for the column tiles; mirror
blocks aggregate in SBUF per target tile j and get the same hierarchical
pair-max top-8 treatment. Leftover tile 48 gets a 4096-wide window split
512/core, with its own transposes.

PE work: (6*3328 + 512) cols x 8 groups = 164k cyc + ~21k transpose cyc
~= 77us/core, vs 307k cyc (128us) for the full-matrix kernel.

Host: expand group candidates per target tile, exact fp32 rescore,
top-9, drop self, two GCN layers.
"""

import os
import sys

import numpy as np

if "/opt/trn_rl_repo" not in sys.path:
    sys.path.insert(0, "/opt/trn_rl_repo")

B, H, W, C = 32, 14, 14, 2048
N = B * H * W            # 6272 nodes
K = 8
N_CORES = 8
RT = 128
NT = 6                   # own row tiles per core
OWN = NT * RT            # 768
NTILES = N // RT         # 49
WIN = 3328               # forward window (26 blocks)
NBLK = WIN // RT         # 26
GENW = [1536, 1024, 768]
GEN0 = [0, 1536, 2560]
NGEN = 3
NB = 512
SH_WIN = 4096            # tile-48 window
SH_W = SH_WIN // N_CORES  # 512 per core
MAXJ = NT - 1 + NBLK - 1  # 30: highest mirror target (rotated tile idx)
NMIR = MAXJ + 1           # 31 mirror buffers
# cand slots: own-upper 6*3, mirror 31, shared-upper 1, shared-mirror 4,
# plus one extra for tile 0's split final gen
SLOT_UP = 0
SLOT_MIR = NT * NGEN          # 18
SLOT_SHU = SLOT_MIR + NMIR    # 49
SLOT_SHM = SLOT_SHU + 1       # 50
SLOT_T0B = SLOT_SHM + 4       # 54: tile 0 gen-2 second half
NCHUNK = SLOT_T0B + 1         # 55


def _gens(t):
    """Per-tile gen plan: (width, window offset, cand slot)."""
    return [(GENW[k], GEN0[k], SLOT_UP + t * NGEN + k) for k in range(NGEN)]

LAST_EXEC_NS = None
LAST_KNN = None
_PROG = None


def _mir_trange(j):
    return max(0, j - (NBLK - 1)), min(NT - 1, j)


def _build_program():
    from concourse import bacc, tile, mybir

    f32 = mybir.dt.float32
    bf16 = mybir.dt.bfloat16
    f8 = mybir.dt.float8e4
    u16 = mybir.dt.uint16

    nc = bacc.Bacc("TRN2", target_bir_lowering=False)
    x8 = nc.declare_dram_parameter("x8", [8, 128, 2, N], f8, isOutput=False)
    xr7d = nc.declare_dram_parameter("xr7", [128, 2, OWN], f8, isOutput=False)
    xshd = nc.declare_dram_parameter("xsh", [128, 2, 8, RT], f8, isOutput=False)
    xmvd = nc.declare_dram_parameter("xmv", [128, 2, 8, SH_W], f8, isOutput=False)
    eyed = nc.declare_dram_parameter("eye", [128, 128], bf16, isOutput=False)
    cand = nc.declare_dram_parameter("cand", [NCHUNK, 128, 16], u16, isOutput=True)

    Act = mybir.ActivationFunctionType
    DR = mybir.MatmulPerfMode.DoubleRow
    MAX = mybir.AluOpType.max
    KP = 8

    with tile.TileContext(nc) as tc:
        with (
            tc.tile_pool(name="persist", bufs=1) as pp,
            tc.tile_pool(name="score", bufs=3) as scp,
            tc.tile_pool(name="p1", bufs=3) as p1p,
            tc.tile_pool(name="p2", bufs=3) as p2p,
            tc.tile_pool(name="stage", bufs=10) as sp,
            tc.tile_pool(name="psum", bufs=2, space="PSUM") as psp,
            tc.tile_pool(name="ptr", bufs=2, space="PSUM") as ptp,
        ):
            XCOLS = (NT - 1) * RT + WIN              # 3968 cols ever touched
            xs = [pp.tile([128, 2, XCOLS], f8, name=f"xs{kp}")
                  for kp in range(KP)]
            xr7 = pp.tile([128, 2, OWN], f8)
            xsh = pp.tile([128, 2, KP, RT], f8)
            xmv = pp.tile([128, 2, KP, SH_W], f8)
            eye = pp.tile([128, 128], bf16)
            ssh = pp.tile([128, SH_W], bf16)
            M = pp.tile([128, NMIR, OWN], bf16)      # mirror aggregation
            Msh = pp.tile([128, 4, 128], bf16)       # shared-tile mirrors

            # single sync HW-DGE queue for bulk loads (concurrent streams
            # contend with PE SBUF fetches); first slab per group first
            for kp in range(KP - 1):
                nc.sync.dma_start(out=xs[kp][:, :, 0:GENW[0]],
                                  in_=x8[kp, :, :, 0:GENW[0]])
            nc.sync.dma_start(out=xr7[:], in_=xr7d[:])
            nc.sync.dma_start(out=xs[KP - 1][:, :, 0:GENW[0]],
                              in_=x8[KP - 1, :, :, 0:GENW[0]])
            # sweep-0 extension: unit (t,0) reads up to col 128t+1536, so the
            # whole first sweep needs [0:2176) — load the tail right away or
            # units 1..5 stall mid-sweep
            for kp in range(KP):
                nc.sync.dma_start(out=xs[kp][:, :, 1536:2176],
                                  in_=x8[kp, :, :, 1536:2176])
            nc.sync.dma_start(out=eye[:], in_=eyed[:])
            nc.sync.dma_start(out=xmv[:], in_=xmvd[:])
            nc.sync.dma_start(out=xsh[:], in_=xshd[:])
            # remaining columns: window of last tile reaches 128*5+3328=3968;
            # shared window is served by xmv
            for c0, c1 in [(2176, 3072), (3072, 3968)]:
                for kp in range(KP):
                    nc.sync.dma_start(out=xs[kp][:, :, c0:c1],
                                      in_=x8[kp, :, :, c0:c1])

            def topk_chunk(src_ap, slot):
                stage = sp.tile([128, 16], u16, tag="st")
                nc.vector.max(stage[:, 0:8].bitcast(bf16), src_ap)
                nc.vector.max_index(stage[:, 8:16], stage[:, 0:8].bitcast(bf16),
                                    src_ap)
                nc.sync.dma_start(out=cand[slot], in_=stage[:, :])

            def hier_topk(src, width, slot, eng=None):
                """pair-max halving x2 then top-8 of width/4 groups.

                eng=nc.gpsimd offloads the halving to the idle Pool engine
                (used for mirror chains, which bunch near the end of the
                run when the DVE is busiest)."""
                eng = eng or nc.vector
                hf, qt = width // 2, width // 4
                p1 = p1p.tile([128, 768], bf16, tag="p1")
                p2 = p2p.tile([128, 384], bf16, tag="p2")
                eng.tensor_tensor(p1[:, 0:hf], src[:, 0:hf],
                                  src[:, hf:width], MAX)
                eng.tensor_tensor(p2[:, 0:qt], p1[:, 0:qt],
                                  p1[:, qt:hf], MAX)
                topk_chunk(p2[:, 0:qt], slot)

            mir_done = [0] * NMIR     # contributed-tile count per buffer

            def emit_transposes(t, g0, width, s):
                """Transpose a gen's 128-col blocks into mirror slots."""
                m0 = g0 // RT
                nblk = width // RT
                done = []
                for b0 in range(0, nblk, 8):
                    bn = min(8, nblk - b0)
                    pt = ptp.tile([128, 1024], bf16, tag="pt")
                    for k in range(bn):
                        nc.tensor.transpose(
                            pt[:, k * 128:(k + 1) * 128],
                            s[:, (b0 + k) * RT:(b0 + k + 1) * RT],
                            eye[:],
                        )
                    j0 = t + m0 + b0
                    nc.scalar.activation(
                        M[:, j0:j0 + bn, t * RT:(t + 1) * RT],
                        pt[:, 0:bn * 128], Act.Copy)
                    done.extend(range(j0, j0 + bn))
                return done

            def flush_mirrors(newly):
                for j in newly:
                    mir_done[j] += 1
                for j in range(NMIR):
                    tmin, tmax = _mir_trange(j)
                    if mir_done[j] == tmax - tmin + 1:
                        mir_done[j] = -99  # emitted
                        hier_topk(M[:, j, tmin * RT:(tmax + 1) * RT],
                                  (tmax - tmin + 1) * RT, SLOT_MIR + j)

            def own_gen(t, width, g0, slot):
                r0 = t * RT
                w0 = r0 + g0                       # window offset in rotated cols
                ps = psp.tile([128, 1536], f32, tag="ps", name=f"ps_{t}_{g0}")
                for kp in range(KP):
                    lhsT = (xs[kp][:, :, r0:r0 + RT] if kp < KP - 1
                            else xr7[:, :, r0:r0 + RT])
                    for j in range(0, width, NB):
                        jw = min(NB, width - j)
                        nc.tensor.matmul(
                            ps[:, j:j + jw], lhsT,
                            xs[kp][:, :, w0 + j:w0 + j + jw],
                            start=(kp == 0), stop=(kp == KP - 1),
                            perf_mode=DR, skip_group_check=True,
                        )
                s = scp.tile([128, 1536], bf16, tag="s", name=f"s_{t}_{g0}")
                nc.scalar.activation(s[:, 0:width], ps[:, 0:width], Act.Copy)
                hier_topk(s, width, slot)
                return s

            def shared_compute():
                """tile 48: this core's 512-col slice (matmul + upper topk)."""
                ps = psp.tile([128, 1536], f32, tag="ps", name="ps_sh")
                for kp in range(KP):
                    nc.tensor.matmul(
                        ps[:, 0:SH_W], xsh[:, :, kp, :], xmv[:, :, kp, :],
                        start=(kp == 0), stop=(kp == KP - 1),
                        perf_mode=DR, skip_group_check=True,
                    )
                nc.scalar.activation(ssh[:, :], ps[:, 0:SH_W], Act.Copy)
                hier_topk(ssh, SH_W, SLOT_SHU)

            def shared_transposes():
                pt = ptp.tile([128, 1024], bf16, tag="pt")
                for k in range(4):
                    nc.tensor.transpose(pt[:, k * 128:(k + 1) * 128],
                                        ssh[:, k * 128:(k + 1) * 128], eye[:])
                nc.scalar.activation(Msh[:, :, :], pt[:, 0:512], Act.Copy)
                for k in range(4):
                    hier_topk(Msh[:, k, :], 128, SLOT_SHM + k)

            # gen-major sweeps; transposes delayed one unit so the PE never
            # waits on the score drain. Last sweep runs t descending so the
            # final mirror buffers (which need tile 0's last blocks) finish
            # while earlier buffers' chains still overlap compute; tile 0's
            # split last gen halves the chain pile after the final unit.
            units = []
            for g in range(NGEN - 1):
                for t in range(NT):
                    units.append((t,) + _gens(t)[g][:2] + (_gens(t)[g][2],))
            for t in reversed(range(NT)):
                units.append((t,) + _gens(t)[2][:2] + (_gens(t)[2][2],))
            prev = None
            for n, (t, width, g0, slot) in enumerate(units):
                s = own_gen(t, width, g0, slot)
                if prev is not None:
                    flush_mirrors(emit_transposes(*prev))
                if (t, g0) == (NT - 1, 0):
                    shared_compute()
                if (t, g0) == (0, GEN0[1]):
                    shared_transposes()
                prev = (t, g0, width, s)

            flush_mirrors(emit_transposes(*prev))
    nc.compile()
    return nc


def _knn_from_device(x_flat):
    global LAST_EXEC_NS, LAST_KNN, _PROG
    import ml_dtypes
    from concourse.bass_utils import run_bass_kernel_spmd

    if _PROG is None:
        _PROG = _build_program()

    xq8 = x_flat.astype(ml_dtypes.float8_e4m3)
    sq = np.sum(x_flat * x_flat, axis=1, dtype=np.float32)
    nhc = -0.5 * (sq - sq.mean())
    a = nhc.astype(ml_dtypes.float8_e4m3)
    bres = (nhc - a.astype(np.float32)).astype(ml_dtypes.float8_e4m3)
    x8T = np.ascontiguousarray(xq8.T)
    x8 = np.ascontiguousarray(
        x8T.reshape(8, 2, 128, N).transpose(0, 2, 1, 3))     # [kp, p, i, n]
    one8 = np.float32(1.0).astype(ml_dtypes.float8_e4m3)
    # moving side channels 2044..2047 = (1, 1, a_j, b_j)
    x8[7, 124, 1, :] = one8
    x8[7, 125, 1, :] = one8
    x8[7, 126, 1, :] = a
    x8[7, 127, 1, :] = bres

    def stat_norm(panel, rows):
        """stationary channels 2044..2047 = (a_i, b_i, 1, 1)."""
        panel[124, 1, :] = a[rows]
        panel[125, 1, :] = bres[rows]
        panel[126, 1, :] = one8
        panel[127, 1, :] = one8

    xsh = np.ascontiguousarray(x8[:, :, :, N_CORES * OWN:N])  # [kp, p, i, 128]
    stat_norm(xsh[7], np.arange(N_CORES * OWN, N))
    xsh_in = np.ascontiguousarray(xsh.transpose(1, 2, 0, 3))  # [p,i,kp,n]

    eye = np.eye(128, dtype=ml_dtypes.bfloat16)

    in_maps = []
    for c in range(N_CORES):
        sh = c * OWN
        x8c = np.ascontiguousarray(np.roll(x8, -sh, axis=3))
        xr7 = np.ascontiguousarray(x8c[7][:, :, 0:OWN])
        stat_norm(xr7, (np.arange(OWN) + sh) % N)
        # shared moving slice: global cols [6144+512c, 6144+512c+512) mod N
        cols = (np.arange(SH_W) + N_CORES * OWN + SH_W * c) % N
        xmv = np.ascontiguousarray(
            x8[:, :, :, cols].transpose(1, 2, 0, 3))
        in_maps.append({"x8": x8c, "xr7": xr7, "xsh": xsh_in, "xmv": xmv,
                        "eye": eye})
    res = run_bass_kernel_spmd(
        _PROG, in_maps, list(range(N_CORES)),
        trace=bool(os.environ.get("KNN_TRACE")),
    )
    if res.exec_time_ns is not None:
        LAST_EXEC_NS = res.exec_time_ns

    # ---- decode: gather candidate columns per global row tile ----
    per_tile = [[] for _ in range(NTILES)]   # lists of [128, n] col arrays
    m4 = np.arange(4, dtype=np.int64)
    for c, r in enumerate(res.results):
        o = r["cand"].astype(np.int64)                   # [NCHUNK, 128, 16]
        # own upper: rows = tile 6c+t, cols rotated 128t+g0+group
        for t in range(NT):
            for w, g0, slot in _gens(t):
                idx = o[slot, :, 8:16]
                rot = t * RT + g0 + idx[:, :, None] + m4 * (w // 4)
                gcol = (rot.reshape(128, 32) + c * OWN) % N
                per_tile[(6 * c + t) % NTILES].append(gcol)
        # mirrors: buffer j rows = tile (6c+j) mod 49; cols = own rows
        for j in range(NMIR):
            tmin, tmax = _mir_trange(j)
            wj = (tmax - tmin + 1) * RT
            idx = o[SLOT_MIR + j, :, 8:16]
            w = idx[:, :, None] + m4 * (wj // 4)
            gcol = c * OWN + tmin * RT + w.reshape(128, 32)
            per_tile[(6 * c + j) % NTILES].append(gcol)
        # shared upper: rows = tile 48; cols = window slice
        idx = o[SLOT_SHU, :, 8:16]
        rot = idx[:, :, None] + m4 * (SH_W // 4)
        gcol = (N_CORES * OWN + SH_W * c + rot.reshape(128, 32)) % N
        per_tile[NTILES - 1].append(gcol)
        # shared mirrors: block k rows = tile (48 + 4c + k) mod 49
        for k in range(4):
            idx = o[SLOT_SHM + k, :, 8:16]
            w = idx[:, :, None] + m4 * 32
            gcol = N_CORES * OWN + w.reshape(128, 32)
            per_tile[(NTILES - 1 + 4 * c + k) % NTILES].append(gcol)

    knn = np.empty((N, K), dtype=np.int64)
    for T in range(NTILES):
        cidx = np.concatenate(per_tile[T], axis=1)       # [128, Tt]
        rows = np.arange(T * RT, (T + 1) * RT)
        xc = x_flat[cidx]                                # [128, Tt, C]
        ex = np.einsum("bc,bkc->bk", x_flat[rows], xc,
                       dtype=np.float32) - 0.5 * sq[cidx]
        order = np.argsort(-ex, axis=1, kind="stable")[:, :3 * (K + 1)]
        top = np.take_along_axis(cidx, order, axis=1)
        for i in range(RT):
            tt = top[i]
            tt = tt[tt != rows[i]]
            _, ui = np.unique(tt, return_index=True)
            tt = tt[np.sort(ui)]
            knn[rows[i]] = tt[:K]
    LAST_KNN = knn
    return knn


def kernel(x, W1, b1, W2, b2):
    x = np.asarray(x, dtype=np.float32)
    W1 = np.asarray(W1, dtype=np.float32)
    b1 = np.asarray(b1, dtype=np.float32)
    W2 = np.asarray(W2, dtype=np.float32)
    b2 = np.asarray(b2, dtype=np.float32)

    xf = x.reshape(N, C)
    knn = _knn_from_device(xf)

    src = np.repeat(np.arange(N, dtype=np.int64), K)
    dst = knn.reshape(-1)
    loops = np.arange(N, dtype=np.int64)
    src = np.concatenate([src, loops])
    dst = np.concatenate([dst, loops])

    deg = np.bincount(dst, minlength=N).astype(np.float32)
    dinv = 1.0 / np.sqrt(np.maximum(deg, 1.0))
    norm = (dinv[src] * dinv[dst]).astype(np.float32)

    try:
        import scipy.sparse as sps
        A = sps.csr_matrix((norm, (dst, src)), shape=(N, N), dtype=np.float32)

        def agg(hw):
            return A @ hw
    except Exception:
        def agg(hw):
            out = np.zeros_like(hw)
            np.add.at(out, dst, hw[src] * norm[:, None])
            return out

    h1 = np.maximum(agg(xf @ W1) + b1, 0.0).astype(np.float32)
    h2 = np.maximum(agg(h1 @ W2) + b2, 0.0).astype(np.float32)
    return h2.reshape(B, H, W, W2.shape[1]).astype(np.float32)
